# revision 37
# baseline (speedup 1.0000x reference)
"""Trainium2 Bass kernel for the attention-gated GRU layer (B=8, N=2048, C=64, Hd=128).

Data-parallel over batch: each of the 8 NeuronCores computes one batch element.
All layouts inside the kernel are "transposed" ([feature, N]) so that the
softmax row dimension lands on the SBUF free axis.

Per-core pipeline:
  X^T, H^T        : PE transposes of the DMA'd inputs
  a1x = W_ha1 X^T : [128, 2048]   (lhs^T of the attention bilinear form)
  a2h = W_ha2 H^T + b_ha2         (rhs^T)
  S^T[m,n]        = a2h[:,m]^T a1x[:,n]    (16 m-tiles, f32r matmuls)
  E^T             = exp(leaky_relu(S^T))   bf16 (leaky = 1-pass custom DVE op)
  [A^T; D]        = [X; 1]^T @ E^T  accumulated over m-chunks (bf16 matmul)
  A^T            /= D  (fast reciprocal + PE-outer-product broadcast)
  Rt, Zt          = sigmoid(W_x* A^T + b)
  Hc              = tanh(W_xh X^T + W_hh (Rt*H^T) + b_xh + b_hh)
  Hnew            = Zt*H^T + (1-Zt)*Hc
  y^T             = W_y Hnew + b_y
  y, H            : PE transposes back to [N, feature], DMA out.
"""

import sys

sys.path.insert(0, "/opt/trn_rl_repo")

import numpy as np

import concourse.bass as bass
import concourse.bacc as bacc
import concourse.mybir as mybir
import concourse.tile as tile
from concourse.bass_utils import run_bass_kernel_spmd
from concourse.masks import make_identity
from concourse import dve_ops as _dve_ops
from concourse.dve_spec import Spec, Src0, C0, maxx, lower, _has_src1
from concourse.dve_uop import DveOpSpec

F32 = mybir.dt.float32
F32R = mybir.dt.float32r
F16 = mybir.dt.float16
BF16 = mybir.dt.bfloat16

B, N, C, Hd = 8, 2048, 64, 128
P = 128
NT = N // P  # 16 tiles of 128 along N

AluOp = mybir.AluOpType
ActFn = mybir.ActivationFunctionType

WEIGHT_SPECS = [
    ("W_ha1", [Hd, C]),
    ("W_ha2", [Hd, Hd]), ("b_ha2", [Hd]),
    ("W_xr", [Hd, C]), ("b_xr", [Hd]),
    ("W_xz", [Hd, C]), ("b_xz", [Hd]),
    ("W_xh", [Hd, C]), ("b_xh", [Hd]),
    ("W_hh", [Hd, Hd]), ("b_hh", [Hd]),
    ("W_y", [C, Hd]), ("b_y", [C]),
]


def _r32(ap):
    return ap.bitcast(F32R)


def _f32(ap):
    return ap.bitcast(F32)


# ---- one-pass leaky_relu as a custom DVE op: out = max(x, x*s0) ----
_LEAKY_SPEC = Spec(
    body=maxx(Src0, Src0 * C0),
    reference=lambda in0, in1, s0, s1, imm2: np.maximum(
        in0.astype(np.float32), in0.astype(np.float32) * s0
    ),
)


def _register_leaky():
    name = "TENSOR_LEAKY_ANT"
    for op in _dve_ops.OPS:
        if op.name == name:
            return op
    opcode = _dve_ops._CUSTOM_DVE_ROW_BASE + len(_dve_ops.OPS)
    shas = {}
    for ver in ("v3", "v4"):
        s = DveOpSpec(name=name, opcode=opcode, uops=lower(_LEAKY_SPEC, ver=ver),
                      rd1_en=_has_src1(_LEAKY_SPEC))
        shas[ver] = s.sha(ver)
    op = _dve_ops.DveOp(name, _LEAKY_SPEC, subdim=False, uops_sha=shas)
    _dve_ops.OPS.append(op)
    _dve_ops._SUB_OPCODE_FOR_NAME[name] = opcode
    _dve_ops.CUSTOM_DVE_SPECS[name] = _LEAKY_SPEC
    return op


LEAKY_OP = _register_leaky()


def build_body(tc, H_pre, X_cur, W, y_out, h_out):
    nc = tc.nc
    import contextlib

    ctx = contextlib.ExitStack()
    with ctx:
        persist = ctx.enter_context(tc.tile_pool(name="persist", bufs=1))

        # ---- load weights (emitted after the input DMAs below) ----
        wsb = {}
        _wdma = []
        for i, (name, shape) in enumerate(WEIGHT_SPECS):
            eng = nc.gpsimd
            if len(shape) == 1:
                t = persist.tile([shape[0], 1], F32, tag=f"w_{name}")
                _wdma.append((eng, t, W[name].ap().rearrange("h -> h ()")))
            else:
                t = persist.tile(shape, F32, tag=f"w_{name}")
                _wdma.append((eng, t, W[name].ap()))
            wsb[name] = t

        # ---- stage inputs + build transposes (per-tile DMA for overlap) ----
        stage = ctx.enter_context(tc.tile_pool(name="stage", bufs=1))
        xs = stage.tile([P, NT, C], F32, tag="xs")
        hs = stage.tile([P, NT, Hd], F32, tag="hs")
        for mq in range(4):
            nc.sync.dma_start(
                out=xs[:, mq * 4:(mq + 1) * 4, :],
                in_=X_cur.ap()[mq * 512:(mq + 1) * 512, :].rearrange(
                    "(m p) c -> p m c", p=P))
            nc.scalar.dma_start(
                out=hs[:, mq * 4:(mq + 1) * 4, :],
                in_=H_pre.ap()[mq * 512:(mq + 1) * 512, :].rearrange(
                    "(m p) h -> p m h", p=P))

        # identity + ACT table prefetch BEFORE weight-DMA dispatches so the
        # GPS/ACT sequencers don't serialize them behind 13 DMA dispatches
        ident = persist.tile([P, P], F32, tag="ident")
        make_identity(nc, ident)
        identr = persist.tile([P, P], BF16, tag="identr")
        nc.vector.tensor_copy(out=identr, in_=ident)
        warm = persist.tile([1, 1], F32, tag="warm")
        nc.vector.memset(warm, 0.0)
        nc.scalar.activation(out=warm, in_=warm, func=ActFn.Exp)
        ones_f = persist.tile([1, C], F32, tag="ones_f")
        nc.vector.memset(ones_f, 1.0)
        ones_b = persist.tile([1, C], BF16, tag="ones_b")
        nc.vector.tensor_copy(out=ones_b, in_=ones_f)

        for eng, t, ap in _wdma:
            eng.dma_start(out=t, in_=ap)

        # combined bias for the candidate gate
        bxhh = persist.tile([Hd, 1], F32, tag="bxhh")
        nc.vector.tensor_add(out=bxhh, in0=wsb["b_xh"], in1=wsb["b_hh"])

        xT = persist.tile([C, N], BF16, tag="xT")
        hT = persist.tile([Hd, N], BF16, tag="hT")
        xaug = persist.tile([P, NT, C + 1], BF16, tag="xaug")

        wT = {}
        with tc.tile_pool(name="tp_psum", bufs=2, space="PSUM") as tp_ps:
            for name, shape in WEIGHT_SPECS:
                if len(shape) != 2:
                    continue
                po, pi = shape  # W [po, pi] -> W^T [pi, po]
                tp = tp_ps.tile([P, P], F32, tag="tp")
                nc.tensor.transpose(tp[:pi, :po], wsb[name][:, :], ident[:po, :po])
                wt = persist.tile([pi, po], BF16, tag=f"wT_{name}")
                nc.vector.tensor_copy(out=wt, in_=tp[:pi, :po])
                wT[name] = wt

            for mq in range(4):
                tpx = tp_ps.tile([P, 4, P], F32, tag="tpx")
                tph = tp_ps.tile([P, 4, P], F32, tag="tph")
                for j in range(4):
                    m = mq * 4 + j
                    nc.tensor.transpose(tpx[:C, j, :], xs[:, m, :], ident)
                    nc.tensor.transpose(tph[:, j, :], hs[:, m, :], ident)
                nc.vector.tensor_copy(out=xT[:, mq * 512:(mq + 1) * 512],
                                      in_=tpx[:C, :, :].rearrange("c a b -> c (a b)"))
                nc.scalar.copy(out=hT[:, mq * 512:(mq + 1) * 512],
                               in_=tph.rearrange("c a b -> c (a b)"))
            nc.vector.tensor_copy(out=xaug[:, :, :C], in_=xs)
            nc.vector.memset(xaug[:, :, C:C + 1], 1.0)

        # ---- attention pre-projections ----
        a1x = persist.tile([Hd, N], BF16, tag="a1x")
        a2h = persist.tile([Hd, N], BF16, tag="a2h")
        with tc.tile_pool(name="mm_psum", bufs=4, space="PSUM") as mm_ps:
            for nb in range(4):
                sl = slice(nb * 512, (nb + 1) * 512)
                ps1 = mm_ps.tile([Hd, 512], F32, tag="mm")
                nc.tensor.matmul(ps1, wT["W_ha1"], xT[:, sl], start=True, stop=True)
                nc.scalar.copy(out=a1x[:, sl], in_=ps1)
                ps2 = mm_ps.tile([Hd, 512], F32, tag="mm")
                nc.tensor.matmul(ps2, wT["W_ha2"], hT[:, sl], start=True, stop=True)
                nc.vector.tensor_scalar_add(out=a2h[:, sl], in0=ps2,
                                            scalar1=wsb["b_ha2"])

        # ---- attention scores + exp + A accumulation ----
        eT = persist.tile([P, NT, N], BF16, tag="eT")
        a_ps_cm = tc.tile_pool(name="a_psum", bufs=1, space="PSUM")
        a_ps = a_ps_cm.__enter__()
        psum_a = a_ps.tile([C + 1, N], F32, tag="acc")
        with tc.tile_pool(name="s_psum", bufs=2, space="PSUM") as s_ps, \
             tc.tile_pool(name="lk", bufs=3) as lk_pool:
            def a_mms(m):
                for nb in range(4):
                    nsl = slice(nb * 512, (nb + 1) * 512)
                    nc.tensor.matmul(psum_a[:, nsl], xaug[:, m, :], eT[:, m, nsl],
                                     start=(m == 0), stop=(m == NT - 1))

            for m in range(NT):
                lkb = lk_pool.tile([P, N], BF16, tag="lkb")
                for h2 in range(2):
                    ps_s = s_ps.tile([P, 1024], F32, tag="s")
                    for q in range(2):
                        nsl = slice(h2 * 1024 + q * 512, h2 * 1024 + (q + 1) * 512)
                        nc.tensor.matmul(ps_s[:, q * 512:(q + 1) * 512],
                                         a2h[:, m * P:(m + 1) * P],
                                         a1x[:, nsl], start=True, stop=True)
                    if 2 * m + h2 in (13, 27):
                        nc.scalar.activation(
                            out=lkb[:, h2 * 1024:(h2 + 1) * 1024], in_=ps_s,
                            func=ActFn.Prelu, alpha=0.01)
                    else:
                        nc.vector._custom_dve(
                            LEAKY_OP, out=lkb[:, h2 * 1024:(h2 + 1) * 1024],
                            in0=ps_s, s0=0.01)
                nc.scalar.activation(out=eT[:, m, :], in_=lkb, func=ActFn.Exp)
                if m > 0:
                    a_mms(m - 1)
            a_mms(NT - 1)

        # ---- softmax normalize + GRU tail (fused, 2 column blocks) ----
        # Pull the sigmoid table-set load forward: overlaps the divide chain
        # instead of stalling the first real sigmoid (all divide ACT ops are
        # set-agnostic Copies).
        nc.scalar.activation(out=warm, in_=warm, func=ActFn.Sigmoid)

        # Copy A_raw/D out of PSUM early so psum_a's banks free for the tail.
        dD = persist.tile([1, N], F32, tag="dD")
        nc.vector.tensor_copy(out=dD, in_=psum_a[C:C + 1, :])
        araw = persist.tile([C, N], BF16, tag="araw")
        nc.scalar.copy(out=araw, in_=psum_a[:C, :])
        a_ps_cm.__exit__(None, None, None)
        dinv = persist.tile([1, N], F32, tag="dinv")

        aT = persist.tile([C, N], BF16, tag="a1x")
        rt = persist.tile([Hd, N], BF16, tag="rt")
        zt = persist.tile([Hd, N], BF16, tag="zt")
        g = persist.tile([Hd, N], BF16, tag="g")
        hc = persist.tile([Hd, N], BF16, tag="hc")
        hnew = persist.tile([Hd, N], BF16, tag="hnew")
        yT = persist.tile([C, N], BF16, tag="yT")

        with tc.tile_pool(name="mm2_psum", bufs=2, space="PSUM") as mm_ps, \
             tc.tile_pool(name="bc_psum", bufs=1, space="PSUM") as bc_ps, \
             tc.tile_pool(name="bc_sb", bufs=2) as bc_sb, \
             tc.tile_pool(name="ot_psum", bufs=1, space="PSUM") as o_ps, \
             tc.tile_pool(name="out_sb", bufs=2) as o_sb:
            NB = 2
            SL = [slice(nb * 1024, (nb + 1) * 1024) for nb in range(NB)]
            QS = [[slice(nb * 1024 + q * 512, nb * 1024 + (q + 1) * 512)
                   for q in range(2)] for nb in range(NB)]
            ps_g = {}

            dinv16 = bc_sb.tile([1, N], BF16, tag="dinv16")
            for nb in range(NB):
                nc.vector.reciprocal_approx_fast(out=dinv[:, SL[nb]],
                                                 in_=dD[:, SL[nb]])
            for nb in range(NB):
                nc.scalar.copy(out=dinv16[:, SL[nb]], in_=dinv[:, SL[nb]])
            pbs = []
            for nb in range(NB):
                ps_b = bc_ps.tile([C, 1024], F32, tag="bc")
                for q in range(2):
                    nc.tensor.matmul(ps_b[:, q * 512:(q + 1) * 512], ones_b,
                                     dinv16[:, QS[nb][q]], start=True, stop=True)
                pbs.append(ps_b)
            dbs = []
            for nb in range(NB):
                dinvb = bc_sb.tile([C, 1024], F32, tag="dinvb")
                nc.scalar.copy(out=dinvb, in_=pbs[nb])
                dbs.append(dinvb)
            for nb in range(NB):
                nc.vector.tensor_tensor(out=aT[:, SL[nb]], in0=araw[:, SL[nb]],
                                        in1=dbs[nb], op=AluOp.mult)
            for nb in range(NB):
                ps = mm_ps.tile([Hd, 1024], F32, tag="mm2")
                for q in range(2):
                    nc.tensor.matmul(ps[:, q * 512:(q + 1) * 512], wT["W_xr"],
                                     aT[:, QS[nb][q]], start=True, stop=True)
                ps_g[("r", nb)] = ps
            for nb in range(NB):
                nc.scalar.activation(out=rt[:, SL[nb]], in_=ps_g[("r", nb)],
                                     func=ActFn.Sigmoid, bias=wsb["b_xr"])
            for nb in range(NB):
                ps = mm_ps.tile([Hd, 1024], F32, tag="mm2")
                for q in range(2):
                    nc.tensor.matmul(ps[:, q * 512:(q + 1) * 512], wT["W_xz"],
                                     aT[:, QS[nb][q]], start=True, stop=True)
                ps_g[("z", nb)] = ps
            for nb in range(NB):
                nc.scalar.activation(out=zt[:, SL[nb]], in_=ps_g[("z", nb)],
                                     func=ActFn.Sigmoid, bias=wsb["b_xz"])
            for nb in range(NB):
                nc.vector.tensor_tensor(out=g[:, SL[nb]], in0=rt[:, SL[nb]],
                                        in1=hT[:, SL[nb]], op=AluOp.mult)
            for nb in range(NB):
                ps = mm_ps.tile([Hd, 1024], F32, tag="mm2")
                for q in range(2):
                    nc.tensor.matmul(ps[:, q * 512:(q + 1) * 512], wT["W_xh"],
                                     xT[:, QS[nb][q]], start=True, stop=False)
                    nc.tensor.matmul(ps[:, q * 512:(q + 1) * 512], wT["W_hh"],
                                     g[:, QS[nb][q]], start=False, stop=True)
                ps_g[("h", nb)] = ps
            for nb in range(NB):
                nc.scalar.activation(out=hc[:, SL[nb]], in_=ps_g[("h", nb)],
                                     func=ActFn.Tanh, bias=bxhh)
            # Hnew = Hc + Zt*(H_pre - Hc)   (rt/zt blocks reused as scratch)
            for nb in range(NB):
                nc.vector.tensor_tensor(out=rt[:, SL[nb]], in0=hT[:, SL[nb]],
                                        in1=hc[:, SL[nb]], op=AluOp.subtract)
            for nb in range(NB):
                nc.vector.tensor_tensor(out=zt[:, SL[nb]], in0=zt[:, SL[nb]],
                                        in1=rt[:, SL[nb]], op=AluOp.mult)
            for nb in range(NB):
                nc.vector.tensor_tensor(out=hnew[:, SL[nb]], in0=zt[:, SL[nb]],
                                        in1=hc[:, SL[nb]], op=AluOp.add)
            for nb in range(NB):
                ps = mm_ps.tile([Hd, 1024], F32, tag="mm2")
                for q in range(2):
                    nc.tensor.matmul(ps[:C, q * 512:(q + 1) * 512], wT["W_y"],
                                     hnew[:, QS[nb][q]], start=True, stop=True)
                ps_g[("y", nb)] = ps
            for nb in range(NB):
                nc.vector.tensor_scalar_add(out=yT[:, SL[nb]],
                                            in0=ps_g[("y", nb)][:C, :],
                                            scalar1=wsb["b_y"])
            # transpose back, batch-copy, DMA out
            tps = {}
            for nb in range(NB):
                tp = o_ps.tile([P, 8, Hd], BF16, tag="otp")
                tpy = o_ps.tile([P, 8, C], BF16, tag="otpy")
                for j in range(8):
                    m = nb * 8 + j
                    msl = slice(m * P, (m + 1) * P)
                    nc.tensor.transpose(tp[:, j, :], hnew[:, msl], identr)
                    nc.tensor.transpose(tpy[:, j, :], yT[:, msl], identr[:C, :C])
                tps[nb] = (tp, tpy)
            for nb in range(NB):
                tp, tpy = tps[nb]
                ot = o_sb.tile([P, 8, Hd], F32, tag="ot")
                nc.scalar.copy(out=ot, in_=tp)
                oty = o_sb.tile([P, 8, C], F32, tag="oty")
                nc.vector.tensor_copy(out=oty, in_=tpy)
                nc.sync.dma_start(
                    out=h_out.ap()[SL[nb], :].rearrange("(j p) h -> p j h", p=P),
                    in_=ot)
                nc.sync.dma_start(
                    out=y_out.ap()[SL[nb], :].rearrange("(j p) c -> p j c", p=P),
                    in_=oty)


def build():
    nc = bacc.Bacc("TRN2", target_bir_lowering=False, debug=False, num_devices=8)
    H_pre = nc.declare_dram_parameter("H_pre", [N, Hd], F32, isOutput=False)
    X_cur = nc.declare_dram_parameter("X_cur", [N, C], F32, isOutput=False)
    W = {}
    for name, shape in WEIGHT_SPECS:
        W[name] = nc.declare_dram_parameter(name, shape, F32, isOutput=False)
    y_out = nc.declare_dram_parameter("y", [N, C], F32, isOutput=True)
    h_out = nc.declare_dram_parameter("H", [N, Hd], F32, isOutput=True)
    with tile.TileContext(nc) as tc:
        build_body(tc, H_pre, X_cur, W, y_out, h_out)
    nc.compile()
    return nc


_NC = None


def kernel(**inputs):
    global _NC
    if _NC is None:
        _NC = build()
    inputs = {k: np.asarray(v, dtype=np.float32) for k, v in inputs.items()}
    in_maps = []
    for i in range(B):
        m = {"H_pre": inputs["H_pre"][i], "X_cur": inputs["X_cur"][i]}
        for name, _ in WEIGHT_SPECS:
            m[name] = inputs[name]
        in_maps.append(m)
    res = run_bass_kernel_spmd(_NC, in_maps, core_ids=list(range(B)))
    y = np.stack([res.results[i]["y"] for i in range(B)])
    H = np.stack([res.results[i]["H"] for i in range(B)])
    return (y, H)


# revision 38
# speedup vs baseline: 1.0401x; 1.0401x over previous
"""Trainium2 Bass kernel for the attention-gated GRU layer (B=8, N=2048, C=64, Hd=128).

Data-parallel over batch: each of the 8 NeuronCores computes one batch element.
All layouts inside the kernel are "transposed" ([feature, N]) so that the
softmax row dimension lands on the SBUF free axis.

Per-core pipeline:
  X^T, H^T        : PE transposes of the DMA'd inputs
  a1x = W_ha1 X^T : [128, 2048]   (lhs^T of the attention bilinear form)
  a2h = W_ha2 H^T + b_ha2         (rhs^T)
  S^T[m,n]        = a2h[:,m]^T a1x[:,n]    (16 m-tiles, f32r matmuls)
  E^T             = exp(leaky_relu(S^T))   bf16 (leaky = 1-pass custom DVE op)
  [A^T; D]        = [X; 1]^T @ E^T  accumulated over m-chunks (bf16 matmul)
  A^T            /= D  (fast reciprocal + PE-outer-product broadcast)
  Rt, Zt          = sigmoid(W_x* A^T + b)
  Hc              = tanh(W_xh X^T + W_hh (Rt*H^T) + b_xh + b_hh)
  Hnew            = Zt*H^T + (1-Zt)*Hc
  y^T             = W_y Hnew + b_y
  y, H            : PE transposes back to [N, feature], DMA out.
"""

import sys

sys.path.insert(0, "/opt/trn_rl_repo")

import numpy as np

import concourse.bass as bass
import concourse.bacc as bacc
import concourse.mybir as mybir
import concourse.tile as tile
from concourse.bass_utils import run_bass_kernel_spmd
from concourse.masks import make_identity
from concourse import dve_ops as _dve_ops
from concourse.dve_spec import Spec, Src0, C0, maxx, lower, _has_src1
from concourse.dve_uop import DveOpSpec

F32 = mybir.dt.float32
F32R = mybir.dt.float32r
F16 = mybir.dt.float16
BF16 = mybir.dt.bfloat16

B, N, C, Hd = 8, 2048, 64, 128
P = 128
NT = N // P  # 16 tiles of 128 along N

AluOp = mybir.AluOpType
ActFn = mybir.ActivationFunctionType

WEIGHT_SPECS = [
    ("W_ha1", [Hd, C]),
    ("W_ha2", [Hd, Hd]), ("b_ha2", [Hd]),
    ("W_xr", [Hd, C]), ("b_xr", [Hd]),
    ("W_xz", [Hd, C]), ("b_xz", [Hd]),
    ("W_xh", [Hd, C]), ("b_xh", [Hd]),
    ("W_hh", [Hd, Hd]), ("b_hh", [Hd]),
    ("W_y", [C, Hd]), ("b_y", [C]),
]


def _r32(ap):
    return ap.bitcast(F32R)


def _f32(ap):
    return ap.bitcast(F32)


# ---- one-pass leaky_relu as a custom DVE op: out = max(x, x*s0) ----
_LEAKY_SPEC = Spec(
    body=maxx(Src0, Src0 * C0),
    reference=lambda in0, in1, s0, s1, imm2: np.maximum(
        in0.astype(np.float32), in0.astype(np.float32) * s0
    ),
)


def _register_leaky():
    name = "TENSOR_LEAKY_ANT"
    for op in _dve_ops.OPS:
        if op.name == name:
            return op
    opcode = _dve_ops._CUSTOM_DVE_ROW_BASE + len(_dve_ops.OPS)
    shas = {}
    for ver in ("v3", "v4"):
        s = DveOpSpec(name=name, opcode=opcode, uops=lower(_LEAKY_SPEC, ver=ver),
                      rd1_en=_has_src1(_LEAKY_SPEC))
        shas[ver] = s.sha(ver)
    op = _dve_ops.DveOp(name, _LEAKY_SPEC, subdim=False, uops_sha=shas)
    _dve_ops.OPS.append(op)
    _dve_ops._SUB_OPCODE_FOR_NAME[name] = opcode
    _dve_ops.CUSTOM_DVE_SPECS[name] = _LEAKY_SPEC
    return op


LEAKY_OP = _register_leaky()


def build_body(tc, H_pre, X_cur, W, y_out, h_out):
    nc = tc.nc
    import contextlib

    ctx = contextlib.ExitStack()
    with ctx:
        persist = ctx.enter_context(tc.tile_pool(name="persist", bufs=1))

        # ---- load weights (emitted after the input DMAs below) ----
        wsb = {}
        _wdma = []
        for i, (name, shape) in enumerate(WEIGHT_SPECS):
            eng = nc.gpsimd
            if len(shape) == 1:
                t = persist.tile([shape[0], 1], F32, tag=f"w_{name}")
                _wdma.append((eng, t, W[name].ap().rearrange("h -> h ()")))
            else:
                t = persist.tile(shape, F32, tag=f"w_{name}")
                _wdma.append((eng, t, W[name].ap()))
            wsb[name] = t

        # ---- stage inputs + build transposes (per-tile DMA for overlap) ----
        stage = ctx.enter_context(tc.tile_pool(name="stage", bufs=1))
        xs = stage.tile([P, NT, C], F32, tag="xs")
        hs = stage.tile([P, NT, Hd], F32, tag="hs")
        for mq in range(4):
            nc.sync.dma_start(
                out=xs[:, mq * 4:(mq + 1) * 4, :],
                in_=X_cur.ap()[mq * 512:(mq + 1) * 512, :].rearrange(
                    "(m p) c -> p m c", p=P))
            nc.scalar.dma_start(
                out=hs[:, mq * 4:(mq + 1) * 4, :],
                in_=H_pre.ap()[mq * 512:(mq + 1) * 512, :].rearrange(
                    "(m p) h -> p m h", p=P))

        # identity + ACT table prefetch BEFORE weight-DMA dispatches so the
        # GPS/ACT sequencers don't serialize them behind 13 DMA dispatches
        ident = persist.tile([P, P], F32, tag="ident")
        make_identity(nc, ident)
        identr = persist.tile([P, P], BF16, tag="identr")
        nc.vector.tensor_copy(out=identr, in_=ident)
        warm = persist.tile([1, 1], F32, tag="warm")
        nc.vector.memset(warm, 0.0)
        nc.scalar.activation(out=warm, in_=warm, func=ActFn.Exp)
        ones_f = persist.tile([1, C], F32, tag="ones_f")
        nc.vector.memset(ones_f, 1.0)
        ones_b = persist.tile([1, P], BF16, tag="ones_b")
        nc.vector.memset(ones_b, 1.0)

        for eng, t, ap in _wdma:
            eng.dma_start(out=t, in_=ap)

        # combined bias for the candidate gate
        bxhh = persist.tile([Hd, 1], F32, tag="bxhh")
        nc.vector.tensor_add(out=bxhh, in0=wsb["b_xh"], in1=wsb["b_hh"])

        xT = persist.tile([C, N], BF16, tag="xT")
        hT = persist.tile([Hd, N], BF16, tag="hT")
        xaug = persist.tile([P, NT, C + 1], BF16, tag="xaug")

        wT = {}
        with tc.tile_pool(name="tp_psum", bufs=2, space="PSUM") as tp_ps:
            for name, shape in WEIGHT_SPECS:
                if len(shape) != 2:
                    continue
                po, pi = shape  # W [po, pi] -> W^T [pi, po]
                tp = tp_ps.tile([P, P], F32, tag="tp")
                nc.tensor.transpose(tp[:pi, :po], wsb[name][:, :], ident[:po, :po])
                wt = persist.tile([pi, po], BF16, tag=f"wT_{name}")
                nc.vector.tensor_copy(out=wt, in_=tp[:pi, :po])
                wT[name] = wt

            for mq in range(4):
                tpx = tp_ps.tile([P, 4, P], F32, tag="tpx")
                tph = tp_ps.tile([P, 4, P], F32, tag="tph")
                for j in range(4):
                    m = mq * 4 + j
                    nc.tensor.transpose(tpx[:C, j, :], xs[:, m, :], ident)
                    nc.tensor.transpose(tph[:, j, :], hs[:, m, :], ident)
                nc.vector.tensor_copy(out=xT[:, mq * 512:(mq + 1) * 512],
                                      in_=tpx[:C, :, :].rearrange("c a b -> c (a b)"))
                nc.scalar.copy(out=hT[:, mq * 512:(mq + 1) * 512],
                               in_=tph.rearrange("c a b -> c (a b)"))
            nc.vector.tensor_copy(out=xaug[:, :, :C], in_=xs)
            nc.vector.memset(xaug[:, :, C:C + 1], 1.0)

        # ---- attention pre-projections ----
        a1x = persist.tile([Hd, N], BF16, tag="a1x")
        a2h = persist.tile([Hd, N], BF16, tag="a2h")
        with tc.tile_pool(name="mm_psum", bufs=4, space="PSUM") as mm_ps:
            for nb in range(4):
                sl = slice(nb * 512, (nb + 1) * 512)
                ps1 = mm_ps.tile([Hd, 512], F32, tag="mm")
                nc.tensor.matmul(ps1, wT["W_ha1"], xT[:, sl], start=True, stop=True)
                nc.scalar.copy(out=a1x[:, sl], in_=ps1)
                ps2 = mm_ps.tile([Hd, 512], F32, tag="mm")
                nc.tensor.matmul(ps2, wT["W_ha2"], hT[:, sl], start=True, stop=True)
                nc.vector.tensor_scalar_add(out=a2h[:, sl], in0=ps2,
                                            scalar1=wsb["b_ha2"])

        # ---- attention scores + exp + A accumulation ----
        eT = persist.tile([P, NT, N], BF16, tag="eT")
        a_ps_cm = tc.tile_pool(name="a_psum", bufs=1, space="PSUM")
        a_ps = a_ps_cm.__enter__()
        psum_a = a_ps.tile([C + 1, N], F32, tag="acc")
        with tc.tile_pool(name="s_psum", bufs=2, space="PSUM") as s_ps, \
             tc.tile_pool(name="lk", bufs=3) as lk_pool:
            def a_mms(m):
                for nb in range(4):
                    nsl = slice(nb * 512, (nb + 1) * 512)
                    nc.tensor.matmul(psum_a[:, nsl], xaug[:, m, :], eT[:, m, nsl],
                                     start=(m == 0), stop=(m == NT - 1))

            for m in range(NT):
                lkb = lk_pool.tile([P, N], BF16, tag="lkb")
                for h2 in range(2):
                    ps_s = s_ps.tile([P, 1024], F32, tag="s")
                    for q in range(2):
                        nsl = slice(h2 * 1024 + q * 512, h2 * 1024 + (q + 1) * 512)
                        nc.tensor.matmul(ps_s[:, q * 512:(q + 1) * 512],
                                         a2h[:, m * P:(m + 1) * P],
                                         a1x[:, nsl], start=True, stop=True)
                    if 2 * m + h2 in (13, 27):
                        nc.scalar.activation(
                            out=lkb[:, h2 * 1024:(h2 + 1) * 1024], in_=ps_s,
                            func=ActFn.Prelu, alpha=0.01)
                    else:
                        nc.vector._custom_dve(
                            LEAKY_OP, out=lkb[:, h2 * 1024:(h2 + 1) * 1024],
                            in0=ps_s, s0=0.01)
                nc.scalar.activation(out=eT[:, m, :], in_=lkb, func=ActFn.Exp)
                if m > 0:
                    a_mms(m - 1)
            a_mms(NT - 1)

        # ---- softmax normalize + GRU tail (fused, 2 column blocks) ----
        # Pull the sigmoid table-set load forward: overlaps the divide chain
        # instead of stalling the first real sigmoid (all divide ACT ops are
        # set-agnostic Copies).
        nc.scalar.activation(out=warm, in_=warm, func=ActFn.Sigmoid)

        # Copy A_raw/D out of PSUM early so psum_a's banks free for the tail.
        dD = persist.tile([1, N], F32, tag="dD")
        nc.vector.tensor_copy(out=dD[:, 0:1024], in_=psum_a[C:C + 1, 0:1024])
        araw = persist.tile([C, N], BF16, tag="araw")
        nc.scalar.copy(out=araw, in_=psum_a[:C, :])
        nc.scalar.copy(out=dD[:, 1024:N], in_=psum_a[C:C + 1, 1024:N])
        a_ps_cm.__exit__(None, None, None)
        dinv = persist.tile([1, N], F32, tag="dinv")

        rt = persist.tile([Hd, N], BF16, tag="rt")
        zt = persist.tile([Hd, N], BF16, tag="zt")
        g = persist.tile([Hd, N], BF16, tag="g")
        hc = persist.tile([Hd, N], BF16, tag="hc")
        hnew = persist.tile([Hd, N], BF16, tag="hnew")
        yT = persist.tile([C, N], BF16, tag="yT")

        with tc.tile_pool(name="mm2_psum", bufs=2, space="PSUM") as mm_ps, \
             tc.tile_pool(name="bc_psum", bufs=1, space="PSUM") as bc_ps, \
             tc.tile_pool(name="bc_sb", bufs=2) as bc_sb, \
             tc.tile_pool(name="ot_psum", bufs=1, space="PSUM") as o_ps, \
             tc.tile_pool(name="out_sb", bufs=2) as o_sb:
            NB = 2
            SL = [slice(nb * 1024, (nb + 1) * 1024) for nb in range(NB)]
            QS = [[slice(nb * 1024 + q * 512, nb * 1024 + (q + 1) * 512)
                   for q in range(2)] for nb in range(NB)]
            ps_g = {}

            dinv16 = bc_sb.tile([1, N], BF16, tag="dinv16")
            for nb in range(NB):
                nc.vector.reciprocal_approx_fast(out=dinv[:, SL[nb]],
                                                 in_=dD[:, SL[nb]])
            for nb in range(NB):
                nc.scalar.copy(out=dinv16[:, SL[nb]], in_=dinv[:, SL[nb]])
            # gate matmuls on UNNORMALIZED araw (scaling commutes); they run
            # concurrently with the reciprocal/broadcast chain
            for nb in range(NB):
                ps = mm_ps.tile([Hd, 1024], F32, tag="mm2")
                for q in range(2):
                    nc.tensor.matmul(ps[:, q * 512:(q + 1) * 512], wT["W_xr"],
                                     araw[:, QS[nb][q]], start=True, stop=True)
                ps_g[("r", nb)] = ps
            for nb in range(NB):
                ps = mm_ps.tile([Hd, 1024], F32, tag="mm2")
                for q in range(2):
                    nc.tensor.matmul(ps[:, q * 512:(q + 1) * 512], wT["W_xz"],
                                     araw[:, QS[nb][q]], start=True, stop=True)
                ps_g[("z", nb)] = ps
            pbs = []
            for nb in range(NB):
                ps_b = bc_ps.tile([P, 1024], F32, tag="bc")
                for q in range(2):
                    nc.tensor.matmul(ps_b[:, q * 512:(q + 1) * 512], ones_b,
                                     dinv16[:, QS[nb][q]], start=True, stop=True)
                pbs.append(ps_b)
            dbs = []
            for nb in range(NB):
                dinvb = bc_sb.tile([P, 1024], F32, tag="dinvb")
                nc.scalar.copy(out=dinvb, in_=pbs[nb])
                dbs.append(dinvb)
            gp_r, gp_z = [], []
            for nb in range(NB):
                t = bc_sb.tile([Hd, 1024], BF16, tag="gpr")
                nc.vector.tensor_tensor(out=t, in0=ps_g[("r", nb)],
                                        in1=dbs[nb], op=AluOp.mult)
                gp_r.append(t)
            for nb in range(NB):
                nc.scalar.activation(out=rt[:, SL[nb]], in_=gp_r[nb],
                                     func=ActFn.Sigmoid, bias=wsb["b_xr"])
            for nb in range(NB):
                t = bc_sb.tile([Hd, 1024], BF16, tag="gpz")
                nc.vector.tensor_tensor(out=t, in0=ps_g[("z", nb)],
                                        in1=dbs[nb], op=AluOp.mult)
                gp_z.append(t)
            for nb in range(NB):
                nc.scalar.activation(out=zt[:, SL[nb]], in_=gp_z[nb],
                                     func=ActFn.Sigmoid, bias=wsb["b_xz"])
            for nb in range(NB):
                nc.vector.tensor_tensor(out=g[:, SL[nb]], in0=rt[:, SL[nb]],
                                        in1=hT[:, SL[nb]], op=AluOp.mult)
            for nb in range(NB):
                ps = mm_ps.tile([Hd, 1024], F32, tag="mm2")
                for q in range(2):
                    nc.tensor.matmul(ps[:, q * 512:(q + 1) * 512], wT["W_xh"],
                                     xT[:, QS[nb][q]], start=True, stop=False)
                    nc.tensor.matmul(ps[:, q * 512:(q + 1) * 512], wT["W_hh"],
                                     g[:, QS[nb][q]], start=False, stop=True)
                ps_g[("h", nb)] = ps
            for nb in range(NB):
                nc.scalar.activation(out=hc[:, SL[nb]], in_=ps_g[("h", nb)],
                                     func=ActFn.Tanh, bias=bxhh)
            # Hnew = Hc + Zt*(H_pre - Hc)   (rt/zt blocks reused as scratch)
            for nb in range(NB):
                nc.vector.tensor_tensor(out=rt[:, SL[nb]], in0=hT[:, SL[nb]],
                                        in1=hc[:, SL[nb]], op=AluOp.subtract)
            for nb in range(NB):
                nc.vector.tensor_tensor(out=zt[:, SL[nb]], in0=zt[:, SL[nb]],
                                        in1=rt[:, SL[nb]], op=AluOp.mult)
            for nb in range(NB):
                nc.vector.tensor_tensor(out=hnew[:, SL[nb]], in0=zt[:, SL[nb]],
                                        in1=hc[:, SL[nb]], op=AluOp.add)
            for nb in range(NB):
                ps = mm_ps.tile([Hd, 1024], F32, tag="mm2")
                for q in range(2):
                    nc.tensor.matmul(ps[:C, q * 512:(q + 1) * 512], wT["W_y"],
                                     hnew[:, QS[nb][q]], start=True, stop=True)
                ps_g[("y", nb)] = ps
            for nb in range(NB):
                nc.vector.tensor_scalar_add(out=yT[:, SL[nb]],
                                            in0=ps_g[("y", nb)][:C, :],
                                            scalar1=wsb["b_y"])
            # transpose back, batch-copy, DMA out
            tps = {}
            for nb in range(NB):
                tp = o_ps.tile([P, 8, Hd], BF16, tag="otp")
                tpy = o_ps.tile([P, 8, C], BF16, tag="otpy")
                for j in range(8):
                    m = nb * 8 + j
                    msl = slice(m * P, (m + 1) * P)
                    nc.tensor.transpose(tp[:, j, :], hnew[:, msl], identr)
                    nc.tensor.transpose(tpy[:, j, :], yT[:, msl], identr[:C, :C])
                tps[nb] = (tp, tpy)
            for nb in range(NB):
                tp, tpy = tps[nb]
                ot = o_sb.tile([P, 8, Hd], F32, tag="ot")
                nc.scalar.copy(out=ot, in_=tp)
                oty = o_sb.tile([P, 8, C], F32, tag="oty")
                nc.vector.tensor_copy(out=oty, in_=tpy)
                nc.sync.dma_start(
                    out=h_out.ap()[SL[nb], :].rearrange("(j p) h -> p j h", p=P),
                    in_=ot)
                nc.sync.dma_start(
                    out=y_out.ap()[SL[nb], :].rearrange("(j p) c -> p j c", p=P),
                    in_=oty)


def build():
    nc = bacc.Bacc("TRN2", target_bir_lowering=False, debug=False, num_devices=8)
    H_pre = nc.declare_dram_parameter("H_pre", [N, Hd], F32, isOutput=False)
    X_cur = nc.declare_dram_parameter("X_cur", [N, C], F32, isOutput=False)
    W = {}
    for name, shape in WEIGHT_SPECS:
        W[name] = nc.declare_dram_parameter(name, shape, F32, isOutput=False)
    y_out = nc.declare_dram_parameter("y", [N, C], F32, isOutput=True)
    h_out = nc.declare_dram_parameter("H", [N, Hd], F32, isOutput=True)
    with tile.TileContext(nc) as tc:
        build_body(tc, H_pre, X_cur, W, y_out, h_out)
    nc.compile()
    return nc


_NC = None


def kernel(**inputs):
    global _NC
    if _NC is None:
        _NC = build()
    inputs = {k: np.asarray(v, dtype=np.float32) for k, v in inputs.items()}
    in_maps = []
    for i in range(B):
        m = {"H_pre": inputs["H_pre"][i], "X_cur": inputs["X_cur"][i]}
        for name, _ in WEIGHT_SPECS:
            m[name] = inputs[name]
        in_maps.append(m)
    res = run_bass_kernel_spmd(_NC, in_maps, core_ids=list(range(B)))
    y = np.stack([res.results[i]["y"] for i in range(B)])
    H = np.stack([res.results[i]["H"] for i in range(B)])
    return (y, H)
